# revision 39
# baseline (speedup 1.0000x reference)
"""Fused self-attention (FCSelfAttention) Trainium2 Bass kernel.

Problem: X:[4,2048,512] fp32, W_qkv:[512,1536], W_out:[512,512], b_out:[512]
  qkv = X @ W_qkv ; q,k,v -> heads (B,H=8,N=2048,DH=64)
  scores[n,m] = k_n . q_m * DH**-0.5 ; softmax over m (query axis)
  out[n] = sum_e att[n,e] v[e] ; merge heads ; @ W_out + b_out

Sharding (8 cores): batch x head-group. Core c handles batch b=c//2 and
heads 4g..4g+3 where g=c%2. Each core computes a partial output
projection for its batch; the host sums the two partials per batch and
adds b_out.

Device algorithm (per core), flash-style with scores kept transposed so
the softmax axis lands on the TensorE contraction axis:
  S^T[m,n] = sum_d QT[d,m] KT[d,n]        (m = softmax axis, on partitions)
  P^T = exp(S^T * SCALE)                   (no max subtraction; |S| < 9)
  PV:  lhsT = V_aug[e, 0:65] (col 64 = ones) -> psum[0:64]=out^T, psum[64]=Z

ACT (exp) paces the kernel (~1.2us per 128x1024 chunk, 128 chunks).
v2 scheduling, tuned from the HW trace of v1:
  - deep-pipelined prologue: X cols 0:512 + head-group-0 weight slices
    DMA first; attention starts after a minimal serial prefix (qt/kt
    piece 0 + a few V tiles); all other projection work drains through
    pending queues inside the exp-paced loop.
  - score prefetch: score matmuls for chunk ec+2 are emitted BEFORE
    PV(ec), so the in-order PE queue never head-of-line-blocks the next
    exp at quarter boundaries.
  - per-head [65, N] attention tiles: the PV psum (64 out rows + Z row)
    drains with ONE copy per head per quarter; Z row is read from
    partition 64 of the same tile.
"""

import sys

import numpy as np

_B, _N, _DIM = 4, 2048, 512
_H, _DH = 8, 64
_SCALE = _DH ** -0.5
_NCORES = 8
_HPC = 4              # heads per core
_HL = _HPC * _DH      # 256 local inner dim
_TC = _N // 128       # 16 token chunks
_KC = _DIM // 128     # 4 contraction chunks for projections

_cache = {}


def _emit(tc, xt, wq, wk, wv, wo, out, mybir):
    nc = tc.nc
    dt = mybir.dt
    f32, bf16 = dt.float32, dt.bfloat16
    Exp = mybir.ActivationFunctionType.Exp
    Copy = mybir.ActivationFunctionType.Copy
    Alu = mybir.AluOpType

    from contextlib import ExitStack

    with ExitStack() as ctx:
        weights = ctx.enter_context(tc.tile_pool(name="weights", bufs=1))
        xtp = ctx.enter_context(tc.tile_pool(name="xtp", bufs=1))
        qkp = ctx.enter_context(tc.tile_pool(name="qkp", bufs=1))
        vap = ctx.enter_context(tc.tile_pool(name="vap", bufs=1))
        atp = ctx.enter_context(tc.tile_pool(name="atp", bufs=1))
        ptp = ctx.enter_context(tc.tile_pool(name="ptp", bufs=4))
        zp = ctx.enter_context(tc.tile_pool(name="zp", bufs=2))
        zdp = ctx.enter_context(tc.tile_pool(name="zdp", bufs=2, space="DRAM"))
        outp = ctx.enter_context(tc.tile_pool(name="outp", bufs=1))
        psA = ctx.enter_context(tc.tile_pool(name="psA", bufs=2, space="PSUM"))
        psOp = ctx.enter_context(tc.tile_pool(name="psO", bufs=2, space="PSUM"))
        psB = ctx.enter_context(tc.tile_pool(name="psB", bufs=2, space="PSUM"))

        # ---- input DMAs, in need-by order across 4 engine queues -------
        # Priority A: xt cols 0:512 (all row chunks), q/k weight columns
        # for head-chunk 0, full V weights.  Priority B: the rest of xt,
        # head-chunk-1 q/k columns, wo.
        xt_sb = []
        for kc in range(_KC):
            xt_sb.append(xtp.tile([128, _N], bf16, tag=f"xt{kc}",
                                  name=f"xt{kc}"))
        wq_sb, wk_sb, wv_sb = [], [], []
        for name, lst in (("wq", wq_sb), ("wk", wk_sb), ("wv", wv_sb)):
            for kc in range(_KC):
                lst.append(weights.tile([128, _HL], bf16, tag=f"{name}{kc}",
                                        name=f"{name}{kc}"))
        wo_sb = []
        for h in range(_HPC):
            wo_sb.append(weights.tile([64, _DIM], bf16, tag=f"wo{h}",
                                      name=f"wo{h}"))

        # NOTE: the Scalar (ACT) queue must issue NO DMAs — descriptor
        # writes cost ~650ns each on the issuing queue and ACT is the
        # pacing engine.  Inputs ride sync + gpsimd only.
        qB, qC = nc.sync, nc.gpsimd
        # A: first 512 token-columns of X + head-chunk-0 q/k weight
        # columns + full V weights (everything quarter 0 needs)
        qB.dma_start(xt_sb[0][:, 0:512], xt[0:128, 0:512])
        qC.dma_start(xt_sb[1][:, 0:512], xt[128:256, 0:512])
        qB.dma_start(xt_sb[2][:, 0:512], xt[256:384, 0:512])
        qC.dma_start(xt_sb[3][:, 0:512], xt[384:512, 0:512])
        for kc in range(_KC):
            qB.dma_start(wq_sb[kc][:, 0:128], wq[kc * 128:(kc + 1) * 128, 0:128])
            qC.dma_start(wk_sb[kc][:, 0:128], wk[kc * 128:(kc + 1) * 128, 0:128])
            (qB if kc % 2 else qC).dma_start(
                wv_sb[kc], wv[kc * 128:(kc + 1) * 128, :])
        # B: rest of X (needed from ec4 of quarter 0 onwards), then
        # head-chunk-1 q/k columns and wo
        qB.dma_start(xt_sb[0][:, 512:_N], xt[0:128, 512:_N])
        qC.dma_start(xt_sb[1][:, 512:_N], xt[128:256, 512:_N])
        qB.dma_start(xt_sb[2][:, 512:_N], xt[256:384, 512:_N])
        qC.dma_start(xt_sb[3][:, 512:_N], xt[384:512, 512:_N])
        for kc in range(_KC):
            qB.dma_start(wq_sb[kc][:, 128:_HL],
                         wq[kc * 128:(kc + 1) * 128, 128:_HL])
            qC.dma_start(wk_sb[kc][:, 128:_HL],
                         wk[kc * 128:(kc + 1) * 128, 128:_HL])
        qB.dma_start(wo_sb[0], wo[0:64, :])
        qC.dma_start(wo_sb[1], wo[64:128, :])
        qB.dma_start(wo_sb[2], wo[128:192, :])
        qC.dma_start(wo_sb[3], wo[192:256, :])

        ones11 = weights.tile([1, 1], f32, tag="ones11", name="ones11")
        nc.vector.memset(ones11, 1.0)

        # Warm the PE clock + preload the Exp table while input DMAs land.
        dummy = xtp.tile([128, 512], bf16, tag="dummy", name="dummy")
        nc.vector.memset(dummy, 0.0)
        dumact = xtp.tile([1, 8], bf16, tag="dumact", name="dumact")
        psw = psA.tile([128, 512], f32, tag="mm")
        nc.tensor.matmul(psw, lhsT=dummy[:, 0:128], rhs=dummy,
                         start=True, stop=True)
        nc.scalar.activation(dumact, psw[0:1, 0:8], Exp)
        for _ in range(4):
            psw = psA.tile([128, 512], f32, tag="mm")
            nc.tensor.matmul(psw, lhsT=dummy[:, 0:128], rhs=dummy,
                             start=True, stop=True)

        # ---- qkv projection pieces --------------------------------------
        qt_sb = [None, None]
        kt_sb = [None, None]

        def project_qk_piece(name, wsb, lst, hc, tp, pool=None, tag="mo"):
            if lst[hc] is None:
                lst[hc] = qkp.tile([128, _N], bf16, tag=f"{name}{hc}",
                                   name=f"{name}{hc}")
            dst = lst[hc]
            ps = (pool or psB).tile([128, 512], f32, tag=tag)
            for kc in range(_KC):
                nc.tensor.matmul(
                    ps,
                    lhsT=wsb[kc][:, hc * 128:(hc + 1) * 128],
                    rhs=xt_sb[kc][:, tp * 512:(tp + 1) * 512],
                    start=(kc == 0), stop=(kc == _KC - 1),
                )
            nc.vector.tensor_copy(dst[:, tp * 512:(tp + 1) * 512], ps)

        # V augmented with a ones column: va[t][:, h, 0:64] = V, [..., 64]=1
        va_sb = []
        for t in range(_TC):
            va_sb.append(vap.tile([128, _HPC, 65], bf16, tag=f"va{t}",
                                  name=f"va{t}"))

        def v_piece(t, pool=None, tag="mo"):
            va = va_sb[t]
            nc.gpsimd.memset(va[:, :, 64:65], 1.0)
            ps = (pool or psB).tile([128, _HL], f32, tag=tag)
            for kc in range(_KC):
                nc.tensor.matmul(
                    ps,
                    lhsT=xt_sb[kc][:, t * 128:(t + 1) * 128],
                    rhs=wv_sb[kc],
                    start=(kc == 0), stop=(kc == _KC - 1),
                )
            nc.vector.tensor_copy(
                va[:, :, 0:64], ps.rearrange("p (h d) -> p h d", h=_HPC))

        # Minimum serial prefix before attention: qt0/kt0 piece 0 (the
        # score prime right after these is emitted in the priming block
        # below, before the upfront V pieces, so the first exp does not
        # wait on V).
        V_UPFRONT = 4
        project_qk_piece("qt", wq_sb, qt_sb, 0, 0, pool=psA, tag="mm")
        project_qk_piece("kt", wk_sb, kt_sb, 0, 0, pool=psA, tag="mm")
        for t in range(V_UPFRONT):
            v_piece(t, pool=psA if t % 2 == 0 else psB,
                    tag="mm" if t % 2 == 0 else "mo")

        def mkv(t):
            return lambda: v_piece(t)

        def mkp(name, wsb, lst, hc, tp):
            return lambda: project_qk_piece(name, wsb, lst, hc, tp)

        # pending: drained one op per ec slot (popped BEFORE the score
        # prefetch, so a piece's matmuls always precede the first score
        # that reads it on the in-order PE queue).  Deadlines in quarter
        # 0: qt piece tp by slot 4*tp-2, va[t] by slot t-1, kt tp1 by
        # slot 14; kt tp2/tp3 during later quarters.
        pending = [
            mkp("qt", wq_sb, qt_sb, 0, 1), mkv(4), mkv(5),
            mkp("qt", wq_sb, qt_sb, 0, 2), mkv(6), mkv(7),
            mkp("qt", wq_sb, qt_sb, 0, 3), mkv(8), mkv(9),
            mkp("kt", wk_sb, kt_sb, 0, 1), mkv(10), mkv(11), mkv(12),
            mkv(13), mkv(14), mkv(15),
            mkp("kt", wk_sb, kt_sb, 0, 2),
        ]
        # pair 1 projections + kt tp3, drained at ec%3 during quarters 1-3
        pending_slow = [mkp("kt", wk_sb, kt_sb, 0, 3)]
        for tp in range(_N // 512):
            pending_slow.append(mkp("kt", wk_sb, kt_sb, 1, tp))
            pending_slow.append(mkp("qt", wq_sb, qt_sb, 1, tp))

        # ---- attention (paired heads) with pipelined out-projection ------
        acc = []
        for t in range(_TC):
            acc.append(outp.tile([128, _DIM], f32, tag=f"acc{t}",
                                 name=f"acc{t}"))
        # per (pair, head-in-pair): [65, N] attention rows; row 64 = Z
        at_sb = [[None, None], [None, None]]
        zrec = [None] * _HPC

        def outproj_chunk(pair, t, store, wide=False):
            h0, h1 = 2 * pair, 2 * pair + 1
            tsl = slice(t * 128, (t + 1) * 128)
            ps0 = psB.tile([128, _DIM], f32, tag="mo")
            ps1 = (psOp if wide else psB).tile(
                [128, _DIM], f32, tag="po" if wide else "mo")
            nc.tensor.matmul(ps0, lhsT=at_sb[pair][0][0:64, tsl],
                             rhs=wo_sb[h0], start=True, stop=True)
            nc.tensor.matmul(ps1, lhsT=at_sb[pair][1][0:64, tsl],
                             rhs=wo_sb[h1], start=True, stop=True)
            if h0 == 0:
                nc.vector.tensor_scalar_mul(acc[t], ps0, zrec[h0][:, t:t + 1])
            else:
                nc.vector.scalar_tensor_tensor(
                    out=acc[t], in0=ps0, scalar=zrec[h0][:, t:t + 1],
                    in1=acc[t], op0=Alu.mult, op1=Alu.add,
                )
            nc.vector.scalar_tensor_tensor(
                out=acc[t], in0=ps1, scalar=zrec[h1][:, t:t + 1],
                in1=acc[t], op0=Alu.mult, op1=Alu.add,
            )
            if store:
                (nc.gpsimd if t % 2 else nc.sync).dma_start(
                    out[tsl, :], acc[t])

        NQ = 4                      # n-quarters; po = [65, 512] = 1 bank

        def score_pair(pair, q, ec):
            # two heads' score chunks on disjoint PE row groups; ONE psum
            # tile [128, 1024] (h0 cols 0:512, h1 cols 512:1024)
            ncol = q * 512
            ps = psA.tile([128, 1024], f32, tag="mm")
            nc.tensor.matmul(
                ps[:, 0:512],
                lhsT=qt_sb[pair][0:64, ec * 128:(ec + 1) * 128],
                rhs=kt_sb[pair][0:64, ncol:ncol + 512],
                start=True, stop=True,
            )
            nc.tensor.matmul(
                ps[:, 512:1024],
                lhsT=qt_sb[pair][64:128, ec * 128:(ec + 1) * 128],
                rhs=kt_sb[pair][64:128, ncol:ncol + 512],
                start=True, stop=True,
            )
            return ps

        def mkz(h, zrow, q, qs, eng):
            # 1/Z columns via a DRAM bounce: [1, 512] z row -> [128, 4]
            def zchain():
                zd = zdp.tile([1, 512], f32, tag=f"zd{h % 2}")
                eng.dma_start(zd, zrow[0:1, qs])
                zcol = zp.tile([128, NQ], f32, tag=f"zcol{h % 2}")
                eng.dma_start(
                    zcol, zd.rearrange("o (j p) -> (o p) j", p=128))
                nc.vector.reciprocal(
                    zrec[h][:, q * NQ:(q + 1) * NQ], zcol)
            return zchain

        def mkz_pe(h, zrow, q):
            # tail variant: PE is idle after the last exp; use tensor-
            # engine transposes instead of the DMA bounce
            def zchain():
                pz = psB.tile([128, NQ], f32, tag="mo")
                for j in range(NQ):
                    jj = q * NQ + j
                    nc.tensor.transpose(
                        pz[:, j:j + 1],
                        zrow[0:1, jj * 128:(jj + 1) * 128], ones11)
                nc.vector.reciprocal(
                    zrec[h][:, q * NQ:(q + 1) * NQ], pz)
            return zchain

        ps_q = [None, None]         # prefetched score psums, slots ec%2
        ps_q[0] = score_pair(0, 0, 0)
        ps_q[1] = score_pair(0, 0, 1)
        for pair in range(2):
            if pair == 1:
                while pending_slow:
                    pending_slow.pop(0)()
            h0, h1 = 2 * pair, 2 * pair + 1
            for hi, h in enumerate((h0, h1)):
                at_sb[pair][hi] = atp.tile([64, _N], bf16, tag=f"at{pair}{hi}",
                                           name=f"at{pair}{hi}")
                zrec[h] = zp.tile([128, _TC], f32, tag=f"zrec{h}",
                                  name=f"zrec{h}", bufs=1)
            zrow0 = zp.tile([1, _N], f32, tag=f"zrow{h0}", name=f"zrow{h0}",
                            bufs=1)
            zrow1 = zp.tile([1, _N], f32, tag=f"zrow{h1}", name=f"zrow{h1}",
                            bufs=1)
            for q in range(NQ):
                ncol = q * 512
                qs = slice(ncol, ncol + 512)
                po0 = psOp.tile([65, 512], f32, tag="po")
                po1 = psOp.tile([65, 512], f32, tag="po")
                for ec in range(_TC):
                    if ps_q[ec % 2] is None:
                        ps_q[ec % 2] = score_pair(pair, q, ec)
                    ps = ps_q[ec % 2]
                    pt = ptp.tile([128, 1024], bf16, tag="pt")
                    nc.scalar.activation(pt, ps, Exp, scale=_SCALE)
                    ps_q[ec % 2] = None
                    # deferred work first, so its PE ops precede the
                    # prefetched scores that may depend on them
                    if pending:
                        pending.pop(0)()
                    elif pending_slow and ec % 3 == 1:
                        pending_slow.pop(0)()
                    # prefetch scores 2 chunks ahead (same pair; crosses
                    # into the next quarter / next pair's first quarter)
                    pec = ec + 2
                    if pec < _TC:
                        ps_q[pec % 2] = score_pair(pair, q, pec)
                    elif q + 1 < NQ:
                        ps_q[pec % 2] = score_pair(pair, q + 1, pec - _TC)
                    elif pair == 0:
                        ps_q[pec % 2] = score_pair(1, 0, pec - _TC)
                    nc.tensor.matmul(
                        po0[0:65, :], lhsT=va_sb[ec][:, h0, :],
                        rhs=pt[:, 0:512],
                        start=(ec == 0), stop=(ec == _TC - 1),
                    )
                    nc.tensor.matmul(
                        po1[0:65, :], lhsT=va_sb[ec][:, h1, :],
                        rhs=pt[:, 512:1024],
                        start=(ec == 0), stop=(ec == _TC - 1),
                    )
                # drain the quarter: one [65, 512] copy per head (row 64
                # is the Z row).  In the final quarter ACT is idle after
                # the last exp, so the copies run there.
                last_q = (pair == 1 and q == NQ - 1)
                if last_q:
                    nc.vector.tensor_copy(zrow0[:, qs], po0[64:65, :])
                    nc.scalar.activation(at_sb[pair][0][:, qs],
                                         po0[0:64, :], Copy)
                    nc.vector.tensor_copy(zrow1[:, qs], po1[64:65, :])
                    nc.scalar.activation(at_sb[pair][1][:, qs],
                                         po1[0:64, :], Copy)
                    pending.append(mkz_pe(h0, zrow0, q))
                    pending.append(mkz_pe(h1, zrow1, q))
                else:
                    nc.vector.tensor_copy(at_sb[pair][0][:, qs], po0[0:64, :])
                    nc.vector.tensor_copy(zrow0[:, qs], po0[64:65, :])
                    nc.vector.tensor_copy(at_sb[pair][1][:, qs], po1[0:64, :])
                    nc.vector.tensor_copy(zrow1[:, qs], po1[64:65, :])
                    pending.append(mkz(h0, zrow0, q, qs, nc.sync))
                    pending.append(mkz(h1, zrow1, q, qs, nc.gpsimd))
                    # give the z DMA chains a head start so the out-proj
                    # RMWs don't stall on zrec mid-quarter
                    pending.append(lambda: None)
                    pending.append(lambda: None)
                for j in range(NQ):
                    t = q * NQ + j

                    def mk(pair, t, store, wide):
                        return lambda: outproj_chunk(pair, t, store, wide)

                    pending.append(mk(pair, t, pair == 1, last_q))
        while pending:
            pending.pop(0)()


def _build():
    if "/opt/trn_rl_repo" not in sys.path:
        sys.path.insert(0, "/opt/trn_rl_repo")
    from concourse import bacc, mybir
    import concourse.tile as tile

    dt = mybir.dt
    nc = bacc.Bacc("TRN2", target_bir_lowering=False, debug=False,
                   num_devices=_NCORES)
    xt = nc.dram_tensor("xt", [_DIM, _N], dt.bfloat16, kind="ExternalInput").ap()
    wq = nc.dram_tensor("wq", [_DIM, _HL], dt.bfloat16, kind="ExternalInput").ap()
    wk = nc.dram_tensor("wk", [_DIM, _HL], dt.bfloat16, kind="ExternalInput").ap()
    wv = nc.dram_tensor("wv", [_DIM, _HL], dt.bfloat16, kind="ExternalInput").ap()
    wo = nc.dram_tensor("wo", [_HL, _DIM], dt.bfloat16, kind="ExternalInput").ap()
    out = nc.dram_tensor("out", [_N, _DIM], dt.float32, kind="ExternalOutput").ap()

    with tile.TileContext(nc) as tc:
        _emit(tc, xt, wq, wk, wv, wo, out, mybir)
    nc.compile()
    return nc


def _get_nc():
    if "nc" not in _cache:
        _cache["nc"] = _build()
    return _cache["nc"]


def _shard_inputs(X, W_qkv, W_out):
    import ml_dtypes
    bf16 = ml_dtypes.bfloat16
    in_maps = []
    for c in range(_NCORES):
        b, g = c // 2, c % 2
        cols = slice(g * _HL, (g + 1) * _HL)
        in_maps.append({
            "xt": np.ascontiguousarray(X[b].T).astype(bf16),
            "wq": W_qkv[:, 0 * _DIM:][:, cols].astype(bf16),
            "wk": W_qkv[:, 1 * _DIM:][:, cols].astype(bf16),
            "wv": W_qkv[:, 2 * _DIM:][:, cols].astype(bf16),
            "wo": W_out[g * _HL:(g + 1) * _HL, :].astype(bf16),
        })
    return in_maps


def _run(inputs, trace=False):
    if "/opt/trn_rl_repo" not in sys.path:
        sys.path.insert(0, "/opt/trn_rl_repo")
    from concourse.bass_utils import run_bass_kernel_spmd

    X = np.asarray(inputs["X"], dtype=np.float32)
    W_qkv = np.asarray(inputs["W_qkv"], dtype=np.float32)
    W_out = np.asarray(inputs["W_out"], dtype=np.float32)
    b_out = np.asarray(inputs["b_out"], dtype=np.float32)

    nc = _get_nc()
    in_maps = _shard_inputs(X, W_qkv, W_out)
    res = run_bass_kernel_spmd(nc, in_maps, list(range(_NCORES)), trace=trace)

    out = np.empty((_B, _N, _DIM), dtype=np.float32)
    for b in range(_B):
        out[b] = res.results[2 * b]["out"] + res.results[2 * b + 1]["out"] + b_out
    return out, res.exec_time_ns


def kernel(**inputs) -> np.ndarray:
    out, _ = _run(inputs, trace=False)
    return out


# revision 40
# speedup vs baseline: 1.1260x; 1.1260x over previous
"""Fused self-attention (FCSelfAttention) Trainium2 Bass kernel.

Problem: X:[4,2048,512] fp32, W_qkv:[512,1536], W_out:[512,512], b_out:[512]
  qkv = X @ W_qkv ; q,k,v -> heads (B,H=8,N=2048,DH=64)
  scores[n,m] = k_n . q_m * DH**-0.5 ; softmax over m (query axis)
  out[n] = sum_e att[n,e] v[e] ; merge heads ; @ W_out + b_out

Sharding (8 cores): batch x head-group. Core c handles batch b=c//2 and
heads 4g..4g+3 where g=c%2. Each core computes a partial output
projection for its batch; the host sums the two partials per batch and
adds b_out.

Device algorithm (per core), flash-style with scores kept transposed so
the softmax axis lands on the TensorE contraction axis:
  S^T[m,n] = sum_d QT[d,m] KT[d,n]        (m = softmax axis, on partitions)
  P^T = exp(S^T * SCALE)                   (no max subtraction; |S| < 9)
  PV:  lhsT = V_aug[e, 0:65] (col 64 = ones) -> psum[0:64]=out^T, psum[64]=Z

ACT (exp) paces the kernel (~1.2us per 128x1024 chunk, 128 chunks).
v2 scheduling, tuned from the HW trace of v1:
  - deep-pipelined prologue: X cols 0:512 + head-group-0 weight slices
    DMA first; attention starts after a minimal serial prefix (qt/kt
    piece 0 + a few V tiles); all other projection work drains through
    pending queues inside the exp-paced loop.
  - score prefetch: score matmuls for chunk ec+2 are emitted BEFORE
    PV(ec), so the in-order PE queue never head-of-line-blocks the next
    exp at quarter boundaries.
  - per-head [65, N] attention tiles: the PV psum (64 out rows + Z row)
    drains with ONE copy per head per quarter; Z row is read from
    partition 64 of the same tile.
"""

import sys

import numpy as np

_B, _N, _DIM = 4, 2048, 512
_H, _DH = 8, 64
_SCALE = _DH ** -0.5
_NCORES = 8
_HPC = 4              # heads per core
_HL = _HPC * _DH      # 256 local inner dim
_TC = _N // 128       # 16 token chunks
_KC = _DIM // 128     # 4 contraction chunks for projections

_cache = {}


def _emit(tc, xt, wq, wk, wv, wo, out, mybir):
    nc = tc.nc
    dt = mybir.dt
    f32, bf16 = dt.float32, dt.bfloat16
    Exp = mybir.ActivationFunctionType.Exp
    Copy = mybir.ActivationFunctionType.Copy
    Alu = mybir.AluOpType

    from contextlib import ExitStack

    with ExitStack() as ctx:
        weights = ctx.enter_context(tc.tile_pool(name="weights", bufs=1))
        xtp = ctx.enter_context(tc.tile_pool(name="xtp", bufs=1))
        qkp = ctx.enter_context(tc.tile_pool(name="qkp", bufs=1))
        vap = ctx.enter_context(tc.tile_pool(name="vap", bufs=1))
        atp = ctx.enter_context(tc.tile_pool(name="atp", bufs=1))
        ptp = ctx.enter_context(tc.tile_pool(name="ptp", bufs=4))
        zp = ctx.enter_context(tc.tile_pool(name="zp", bufs=2))
        zdp = ctx.enter_context(tc.tile_pool(name="zdp", bufs=2, space="DRAM"))
        outp = ctx.enter_context(tc.tile_pool(name="outp", bufs=1))
        psA = ctx.enter_context(tc.tile_pool(name="psA", bufs=2, space="PSUM"))
        psOp = ctx.enter_context(tc.tile_pool(name="psO", bufs=2, space="PSUM"))
        psB = ctx.enter_context(tc.tile_pool(name="psB", bufs=2, space="PSUM"))

        # ---- input DMAs, in need-by order across 4 engine queues -------
        # Priority A: xt cols 0:512 (all row chunks), q/k weight columns
        # for head-chunk 0, full V weights.  Priority B: the rest of xt,
        # head-chunk-1 q/k columns, wo.
        xt_sb = []
        for kc in range(_KC):
            xt_sb.append(xtp.tile([128, _N], bf16, tag=f"xt{kc}",
                                  name=f"xt{kc}"))
        wq_sb, wk_sb, wv_sb = [], [], []
        for name, lst in (("wq", wq_sb), ("wk", wk_sb), ("wv", wv_sb)):
            for kc in range(_KC):
                lst.append(weights.tile([128, _HL], bf16, tag=f"{name}{kc}",
                                        name=f"{name}{kc}"))
        wo_sb = []
        for h in range(_HPC):
            wo_sb.append(weights.tile([64, _DIM], bf16, tag=f"wo{h}",
                                      name=f"wo{h}"))

        # NOTE: the Scalar (ACT) queue must issue NO DMAs — descriptor
        # writes cost ~650ns each on the issuing queue and ACT is the
        # pacing engine.  Inputs ride sync + gpsimd only.
        qB, qC = nc.sync, nc.gpsimd
        # A: first 512 token-columns of X + head-chunk-0 q/k weight
        # columns + full V weights (everything quarter 0 needs)
        qB.dma_start(xt_sb[0][:, 0:512], xt[0:128, 0:512])
        qC.dma_start(xt_sb[1][:, 0:512], xt[128:256, 0:512])
        qB.dma_start(xt_sb[2][:, 0:512], xt[256:384, 0:512])
        qC.dma_start(xt_sb[3][:, 0:512], xt[384:512, 0:512])
        for kc in range(_KC):
            qB.dma_start(wq_sb[kc][:, 0:128], wq[kc * 128:(kc + 1) * 128, 0:128])
            qC.dma_start(wk_sb[kc][:, 0:128], wk[kc * 128:(kc + 1) * 128, 0:128])
            (qB if kc % 2 else qC).dma_start(
                wv_sb[kc], wv[kc * 128:(kc + 1) * 128, :])
        # B: rest of X (needed from ec4 of quarter 0 onwards), then
        # head-chunk-1 q/k columns and wo
        qB.dma_start(xt_sb[0][:, 512:_N], xt[0:128, 512:_N])
        qC.dma_start(xt_sb[1][:, 512:_N], xt[128:256, 512:_N])
        qB.dma_start(xt_sb[2][:, 512:_N], xt[256:384, 512:_N])
        qC.dma_start(xt_sb[3][:, 512:_N], xt[384:512, 512:_N])
        for kc in range(_KC):
            qB.dma_start(wq_sb[kc][:, 128:_HL],
                         wq[kc * 128:(kc + 1) * 128, 128:_HL])
            qC.dma_start(wk_sb[kc][:, 128:_HL],
                         wk[kc * 128:(kc + 1) * 128, 128:_HL])
        qB.dma_start(wo_sb[0], wo[0:64, :])
        qC.dma_start(wo_sb[1], wo[64:128, :])
        qB.dma_start(wo_sb[2], wo[128:192, :])
        qC.dma_start(wo_sb[3], wo[192:256, :])

        ones11 = weights.tile([1, 1], f32, tag="ones11", name="ones11")
        nc.vector.memset(ones11, 1.0)

        # Warm the PE clock + preload the Exp table while input DMAs land.
        dummy = xtp.tile([128, 512], bf16, tag="dummy", name="dummy")
        nc.vector.memset(dummy, 0.0)
        dumact = xtp.tile([1, 8], bf16, tag="dumact", name="dumact")
        psw = psA.tile([128, 512], f32, tag="mm")
        nc.tensor.matmul(psw, lhsT=dummy[:, 0:128], rhs=dummy,
                         start=True, stop=True)
        nc.scalar.activation(dumact, psw[0:1, 0:8], Exp)
        for _ in range(4):
            psw = psA.tile([128, 512], f32, tag="mm")
            nc.tensor.matmul(psw, lhsT=dummy[:, 0:128], rhs=dummy,
                             start=True, stop=True)

        # ---- qkv projection pieces --------------------------------------
        qt_sb = [None, None]
        kt_sb = [None, None]

        def project_qk_piece(name, wsb, lst, hc, tp, pool=None, tag="mo"):
            if lst[hc] is None:
                lst[hc] = qkp.tile([128, _N], bf16, tag=f"{name}{hc}",
                                   name=f"{name}{hc}")
            dst = lst[hc]
            ps = (pool or psB).tile([128, 512], f32, tag=tag)
            for kc in range(_KC):
                nc.tensor.matmul(
                    ps,
                    lhsT=wsb[kc][:, hc * 128:(hc + 1) * 128],
                    rhs=xt_sb[kc][:, tp * 512:(tp + 1) * 512],
                    start=(kc == 0), stop=(kc == _KC - 1),
                )
            nc.vector.tensor_copy(dst[:, tp * 512:(tp + 1) * 512], ps)

        # V augmented with a ones column: va[t][:, h, 0:64] = V, [..., 64]=1
        va_sb = []
        for t in range(_TC):
            va_sb.append(vap.tile([128, _HPC, 65], bf16, tag=f"va{t}",
                                  name=f"va{t}"))

        def v_piece(t, pool=None, tag="mo"):
            va = va_sb[t]
            nc.gpsimd.memset(va[:, :, 64:65], 1.0)
            ps = (pool or psB).tile([128, _HL], f32, tag=tag)
            for kc in range(_KC):
                nc.tensor.matmul(
                    ps,
                    lhsT=xt_sb[kc][:, t * 128:(t + 1) * 128],
                    rhs=wv_sb[kc],
                    start=(kc == 0), stop=(kc == _KC - 1),
                )
            nc.vector.tensor_copy(
                va[:, :, 0:64], ps.rearrange("p (h d) -> p h d", h=_HPC))

        # Minimum serial prefix before attention: qt0/kt0 piece 0 (the
        # score prime right after these is emitted in the priming block
        # below, before the upfront V pieces, so the first exp does not
        # wait on V).
        V_UPFRONT = 4
        project_qk_piece("qt", wq_sb, qt_sb, 0, 0, pool=psA, tag="mm")
        project_qk_piece("kt", wk_sb, kt_sb, 0, 0, pool=psA, tag="mm")
        for t in range(V_UPFRONT):
            v_piece(t, pool=psA if t % 2 == 0 else psB,
                    tag="mm" if t % 2 == 0 else "mo")

        def mkv(t):
            return lambda: v_piece(t)

        def mkp(name, wsb, lst, hc, tp):
            return lambda: project_qk_piece(name, wsb, lst, hc, tp)

        # pending: drained one op per ec slot (popped BEFORE the score
        # prefetch, so a piece's matmuls always precede the first score
        # that reads it on the in-order PE queue).  Deadlines in quarter
        # 0: qt piece tp by slot 4*tp-2, va[t] by slot t-1, kt tp1 by
        # slot 14; kt tp2/tp3 during later quarters.
        pending = [
            mkp("qt", wq_sb, qt_sb, 0, 1), mkv(4), mkv(5),
            mkp("qt", wq_sb, qt_sb, 0, 2), mkv(6), mkv(7),
            mkp("qt", wq_sb, qt_sb, 0, 3), mkv(8), mkv(9),
            mkp("kt", wk_sb, kt_sb, 0, 1), mkv(10), mkv(11), mkv(12),
            mkv(13), mkv(14), mkv(15),
            mkp("kt", wk_sb, kt_sb, 0, 2),
        ]
        # pair 1 projections + kt tp3, drained at ec%3 during quarters 1-3
        pending_slow = [mkp("kt", wk_sb, kt_sb, 0, 3)]
        for tp in range(_N // 512):
            pending_slow.append(mkp("kt", wk_sb, kt_sb, 1, tp))
            pending_slow.append(mkp("qt", wq_sb, qt_sb, 1, tp))

        # ---- attention (paired heads) with pipelined out-projection ------
        acc = []
        for t in range(_TC):
            acc.append(outp.tile([128, _DIM], f32, tag=f"acc{t}",
                                 name=f"acc{t}"))
        # per (pair, head-in-pair): [65, N] attention rows; row 64 = Z
        at_sb = [[None, None], [None, None]]
        zrec = [None] * _HPC

        def outproj_chunk(pair, t, store, wide=False):
            h0, h1 = 2 * pair, 2 * pair + 1
            tsl = slice(t * 128, (t + 1) * 128)
            ps0 = psB.tile([128, _DIM], f32, tag="mo")
            ps1 = (psOp if wide else psB).tile(
                [128, _DIM], f32, tag="po" if wide else "mo")
            nc.tensor.matmul(ps0, lhsT=at_sb[pair][0][0:64, tsl],
                             rhs=wo_sb[h0], start=True, stop=True)
            nc.tensor.matmul(ps1, lhsT=at_sb[pair][1][0:64, tsl],
                             rhs=wo_sb[h1], start=True, stop=True)
            if h0 == 0:
                nc.vector.tensor_scalar_mul(acc[t], ps0, zrec[h0][:, t:t + 1])
            else:
                nc.vector.scalar_tensor_tensor(
                    out=acc[t], in0=ps0, scalar=zrec[h0][:, t:t + 1],
                    in1=acc[t], op0=Alu.mult, op1=Alu.add,
                )
            nc.vector.scalar_tensor_tensor(
                out=acc[t], in0=ps1, scalar=zrec[h1][:, t:t + 1],
                in1=acc[t], op0=Alu.mult, op1=Alu.add,
            )
            if store:
                (nc.gpsimd if t % 2 else nc.sync).dma_start(
                    out[tsl, :], acc[t])

        NQ = 4                      # n-quarters; po = [65, 512] = 1 bank

        def score_pair(pair, q, ec):
            # two heads' score chunks on disjoint PE row groups; ONE psum
            # tile [128, 1024] (h0 cols 0:512, h1 cols 512:1024)
            ncol = q * 512
            ps = psA.tile([128, 1024], f32, tag="mm")
            nc.tensor.matmul(
                ps[:, 0:512],
                lhsT=qt_sb[pair][0:64, ec * 128:(ec + 1) * 128],
                rhs=kt_sb[pair][0:64, ncol:ncol + 512],
                start=True, stop=True,
            )
            nc.tensor.matmul(
                ps[:, 512:1024],
                lhsT=qt_sb[pair][64:128, ec * 128:(ec + 1) * 128],
                rhs=kt_sb[pair][64:128, ncol:ncol + 512],
                start=True, stop=True,
            )
            return ps

        def mkz(h, zrow, q, qs, eng):
            # 1/Z columns via a DRAM bounce: [1, 512] z row -> [128, 4]
            def zchain():
                zd = zdp.tile([1, 512], f32, tag=f"zd{h % 2}")
                eng.dma_start(zd, zrow[0:1, qs])
                zcol = zp.tile([128, NQ], f32, tag=f"zcol{h % 2}")
                eng.dma_start(
                    zcol, zd.rearrange("o (j p) -> (o p) j", p=128))
                nc.vector.reciprocal(
                    zrec[h][:, q * NQ:(q + 1) * NQ], zcol)
            return zchain

        def mkz_pe(h, zrow, q):
            # tail variant: PE is idle after the last exp; use tensor-
            # engine transposes instead of the DMA bounce
            def zchain():
                pz = psB.tile([128, NQ], f32, tag="mo")
                for j in range(NQ):
                    jj = q * NQ + j
                    nc.tensor.transpose(
                        pz[:, j:j + 1],
                        zrow[0:1, jj * 128:(jj + 1) * 128], ones11)
                nc.vector.reciprocal(
                    zrec[h][:, q * NQ:(q + 1) * NQ], pz)
            return zchain

        ps_q = [None, None]         # prefetched score psums, slots ec%2
        ps_q[0] = score_pair(0, 0, 0)
        ps_q[1] = score_pair(0, 0, 1)
        for pair in range(2):
            if pair == 1:
                while pending_slow:
                    pending_slow.pop(0)()
            h0, h1 = 2 * pair, 2 * pair + 1
            for hi, h in enumerate((h0, h1)):
                at_sb[pair][hi] = atp.tile([64, _N], bf16, tag=f"at{pair}{hi}",
                                           name=f"at{pair}{hi}")
                zrec[h] = zp.tile([128, _TC], f32, tag=f"zrec{h}",
                                  name=f"zrec{h}", bufs=1)
            zrow0 = zp.tile([1, _N], f32, tag=f"zrow{h0}", name=f"zrow{h0}",
                            bufs=1)
            zrow1 = zp.tile([1, _N], f32, tag=f"zrow{h1}", name=f"zrow{h1}",
                            bufs=1)
            for q in range(NQ):
                ncol = q * 512
                qs = slice(ncol, ncol + 512)
                po0 = psOp.tile([65, 512], f32, tag="po")
                po1 = psOp.tile([65, 512], f32, tag="po")
                for ec in range(_TC):
                    if ps_q[ec % 2] is None:
                        ps_q[ec % 2] = score_pair(pair, q, ec)
                    ps = ps_q[ec % 2]
                    pt = ptp.tile([128, 1024], bf16, tag="pt")
                    nc.scalar.activation(pt, ps, Exp, scale=_SCALE)
                    ps_q[ec % 2] = None
                    # deferred work first, so its PE ops precede the
                    # prefetched scores that may depend on them
                    if pending:
                        pending.pop(0)()
                    elif pending_slow and ec % 3 == 0:
                        pending_slow.pop(0)()
                    # prefetch scores 2 chunks ahead (same pair; crosses
                    # into the next quarter / next pair's first quarter)
                    pec = ec + 2
                    if pec < _TC:
                        ps_q[pec % 2] = score_pair(pair, q, pec)
                    elif q + 1 < NQ:
                        ps_q[pec % 2] = score_pair(pair, q + 1, pec - _TC)
                    elif pair == 0:
                        ps_q[pec % 2] = score_pair(1, 0, pec - _TC)
                    nc.tensor.matmul(
                        po0[0:65, :], lhsT=va_sb[ec][:, h0, :],
                        rhs=pt[:, 0:512],
                        start=(ec == 0), stop=(ec == _TC - 1),
                    )
                    nc.tensor.matmul(
                        po1[0:65, :], lhsT=va_sb[ec][:, h1, :],
                        rhs=pt[:, 512:1024],
                        start=(ec == 0), stop=(ec == _TC - 1),
                    )
                # drain the quarter: one [65, 512] copy per head (row 64
                # is the Z row).  In the final quarter ACT is idle after
                # the last exp, so the copies run there.
                last_q = (pair == 1 and q == NQ - 1)
                if last_q:
                    nc.vector.tensor_copy(zrow0[:, qs], po0[64:65, :])
                    nc.scalar.activation(at_sb[pair][0][:, qs],
                                         po0[0:64, :], Copy)
                    nc.vector.tensor_copy(zrow1[:, qs], po1[64:65, :])
                    nc.scalar.activation(at_sb[pair][1][:, qs],
                                         po1[0:64, :], Copy)
                    pending.append(mkz_pe(h0, zrow0, q))
                    pending.append(mkz_pe(h1, zrow1, q))
                else:
                    nc.vector.tensor_copy(at_sb[pair][0][:, qs], po0[0:64, :])
                    nc.vector.tensor_copy(zrow0[:, qs], po0[64:65, :])
                    nc.vector.tensor_copy(at_sb[pair][1][:, qs], po1[0:64, :])
                    nc.vector.tensor_copy(zrow1[:, qs], po1[64:65, :])
                    pending.append(mkz(h0, zrow0, q, qs, nc.sync))
                    pending.append(mkz(h1, zrow1, q, qs, nc.gpsimd))
                    # give the z DMA chains a head start so the out-proj
                    # RMWs don't stall on zrec mid-quarter
                    pending.append(lambda: None)
                    pending.append(lambda: None)
                for j in range(NQ):
                    t = q * NQ + j

                    def mk(pair, t, store, wide):
                        return lambda: outproj_chunk(pair, t, store, wide)

                    pending.append(mk(pair, t, pair == 1, last_q))
        while pending:
            pending.pop(0)()


def _build():
    if "/opt/trn_rl_repo" not in sys.path:
        sys.path.insert(0, "/opt/trn_rl_repo")
    from concourse import bacc, mybir
    import concourse.tile as tile

    dt = mybir.dt
    nc = bacc.Bacc("TRN2", target_bir_lowering=False, debug=False,
                   num_devices=_NCORES)
    xt = nc.dram_tensor("xt", [_DIM, _N], dt.bfloat16, kind="ExternalInput").ap()
    wq = nc.dram_tensor("wq", [_DIM, _HL], dt.bfloat16, kind="ExternalInput").ap()
    wk = nc.dram_tensor("wk", [_DIM, _HL], dt.bfloat16, kind="ExternalInput").ap()
    wv = nc.dram_tensor("wv", [_DIM, _HL], dt.bfloat16, kind="ExternalInput").ap()
    wo = nc.dram_tensor("wo", [_HL, _DIM], dt.bfloat16, kind="ExternalInput").ap()
    out = nc.dram_tensor("out", [_N, _DIM], dt.float32, kind="ExternalOutput").ap()

    with tile.TileContext(nc) as tc:
        _emit(tc, xt, wq, wk, wv, wo, out, mybir)
    nc.compile()
    return nc


def _get_nc():
    if "nc" not in _cache:
        _cache["nc"] = _build()
    return _cache["nc"]


def _shard_inputs(X, W_qkv, W_out):
    import ml_dtypes
    bf16 = ml_dtypes.bfloat16
    in_maps = []
    for c in range(_NCORES):
        b, g = c // 2, c % 2
        cols = slice(g * _HL, (g + 1) * _HL)
        in_maps.append({
            "xt": np.ascontiguousarray(X[b].T).astype(bf16),
            "wq": W_qkv[:, 0 * _DIM:][:, cols].astype(bf16),
            "wk": W_qkv[:, 1 * _DIM:][:, cols].astype(bf16),
            "wv": W_qkv[:, 2 * _DIM:][:, cols].astype(bf16),
            "wo": W_out[g * _HL:(g + 1) * _HL, :].astype(bf16),
        })
    return in_maps


def _run(inputs, trace=False):
    if "/opt/trn_rl_repo" not in sys.path:
        sys.path.insert(0, "/opt/trn_rl_repo")
    from concourse.bass_utils import run_bass_kernel_spmd

    X = np.asarray(inputs["X"], dtype=np.float32)
    W_qkv = np.asarray(inputs["W_qkv"], dtype=np.float32)
    W_out = np.asarray(inputs["W_out"], dtype=np.float32)
    b_out = np.asarray(inputs["b_out"], dtype=np.float32)

    nc = _get_nc()
    in_maps = _shard_inputs(X, W_qkv, W_out)
    res = run_bass_kernel_spmd(nc, in_maps, list(range(_NCORES)), trace=trace)

    out = np.empty((_B, _N, _DIM), dtype=np.float32)
    for b in range(_B):
        out[b] = res.results[2 * b]["out"] + res.results[2 * b + 1]["out"] + b_out
    return out, res.exec_time_ns


def kernel(**inputs) -> np.ndarray:
    out, _ = _run(inputs, trace=False)
    return out


# revision 42
# speedup vs baseline: 1.1621x; 1.0320x over previous
"""Fused self-attention (FCSelfAttention) Trainium2 Bass kernel.

Problem: X:[4,2048,512] fp32, W_qkv:[512,1536], W_out:[512,512], b_out:[512]
  qkv = X @ W_qkv ; q,k,v -> heads (B,H=8,N=2048,DH=64)
  scores[n,m] = k_n . q_m * DH**-0.5 ; softmax over m (query axis)
  out[n] = sum_e att[n,e] v[e] ; merge heads ; @ W_out + b_out

Sharding (8 cores): batch x head-group. Core c handles batch b=c//2 and
heads 4g..4g+3 where g=c%2. Each core computes a partial output
projection for its batch; the host sums the two partials per batch and
adds b_out.

Device algorithm (per core), flash-style with scores kept transposed so
the softmax axis lands on the TensorE contraction axis:
  S^T[m,n] = sum_d QT[d,m] KT[d,n]        (m = softmax axis, on partitions)
  P^T = exp(S^T * SCALE)                   (no max subtraction; |S| < 9)
  PV:  lhsT = V_aug[e, 0:65] (col 64 = ones) -> psum[0:64]=out^T, psum[64]=Z

ACT (exp) paces the kernel (~1.2us per 128x1024 chunk, 128 chunks).
v2 scheduling, tuned from the HW trace of v1:
  - deep-pipelined prologue: X cols 0:512 + head-group-0 weight slices
    DMA first; attention starts after a minimal serial prefix (qt/kt
    piece 0 + a few V tiles); all other projection work drains through
    pending queues inside the exp-paced loop.
  - score prefetch: score matmuls for chunk ec+2 are emitted BEFORE
    PV(ec), so the in-order PE queue never head-of-line-blocks the next
    exp at quarter boundaries.
  - per-head [65, N] attention tiles: the PV psum (64 out rows + Z row)
    drains with ONE copy per head per quarter; Z row is read from
    partition 64 of the same tile.
"""

import sys

import numpy as np

_B, _N, _DIM = 4, 2048, 512
_H, _DH = 8, 64
_SCALE = _DH ** -0.5
_NCORES = 8
_HPC = 4              # heads per core
_HL = _HPC * _DH      # 256 local inner dim
_TC = _N // 128       # 16 token chunks
_KC = _DIM // 128     # 4 contraction chunks for projections

_cache = {}


def _emit(tc, xt, wq, wk, wv, wo, out, mybir):
    nc = tc.nc
    dt = mybir.dt
    f32, bf16 = dt.float32, dt.bfloat16
    Exp = mybir.ActivationFunctionType.Exp
    Copy = mybir.ActivationFunctionType.Copy
    Alu = mybir.AluOpType

    from contextlib import ExitStack

    with ExitStack() as ctx:
        weights = ctx.enter_context(tc.tile_pool(name="weights", bufs=1))
        xtp = ctx.enter_context(tc.tile_pool(name="xtp", bufs=1))
        qkp = ctx.enter_context(tc.tile_pool(name="qkp", bufs=1))
        vap = ctx.enter_context(tc.tile_pool(name="vap", bufs=1))
        atp = ctx.enter_context(tc.tile_pool(name="atp", bufs=1))
        ptp = ctx.enter_context(tc.tile_pool(name="ptp", bufs=6))
        zp = ctx.enter_context(tc.tile_pool(name="zp", bufs=2))
        zdp = ctx.enter_context(tc.tile_pool(name="zdp", bufs=2, space="DRAM"))
        outp = ctx.enter_context(tc.tile_pool(name="outp", bufs=1))
        psA = ctx.enter_context(tc.tile_pool(name="psA", bufs=2, space="PSUM"))
        psOp = ctx.enter_context(tc.tile_pool(name="psO", bufs=2, space="PSUM"))
        psB = ctx.enter_context(tc.tile_pool(name="psB", bufs=2, space="PSUM"))

        # ---- input DMAs, in need-by order across 4 engine queues -------
        # Priority A: xt cols 0:512 (all row chunks), q/k weight columns
        # for head-chunk 0, full V weights.  Priority B: the rest of xt,
        # head-chunk-1 q/k columns, wo.
        xt_sb = []
        for kc in range(_KC):
            xt_sb.append(xtp.tile([128, _N], bf16, tag=f"xt{kc}",
                                  name=f"xt{kc}"))
        wq_sb, wk_sb, wv_sb = [], [], []
        for name, lst in (("wq", wq_sb), ("wk", wk_sb), ("wv", wv_sb)):
            for kc in range(_KC):
                lst.append(weights.tile([128, _HL], bf16, tag=f"{name}{kc}",
                                        name=f"{name}{kc}"))
        wo_sb = []
        for h in range(_HPC):
            wo_sb.append(weights.tile([64, _DIM], bf16, tag=f"wo{h}",
                                      name=f"wo{h}"))

        # NOTE: the Scalar (ACT) queue must issue NO DMAs — descriptor
        # writes cost ~650ns each on the issuing queue and ACT is the
        # pacing engine.  Inputs ride sync + gpsimd only.
        qB, qC = nc.sync, nc.gpsimd
        # A: head-chunk-0 q/k weight columns first (small; lets the qt/kt
        # projections start the moment X lands), then the first 512
        # token-columns of X, then full V weights
        for kc in range(_KC):
            qB.dma_start(wq_sb[kc][:, 0:128], wq[kc * 128:(kc + 1) * 128, 0:128])
            qC.dma_start(wk_sb[kc][:, 0:128], wk[kc * 128:(kc + 1) * 128, 0:128])
        qB.dma_start(xt_sb[0][:, 0:512], xt[0:128, 0:512])
        qC.dma_start(xt_sb[1][:, 0:512], xt[128:256, 0:512])
        qB.dma_start(xt_sb[2][:, 0:512], xt[256:384, 0:512])
        qC.dma_start(xt_sb[3][:, 0:512], xt[384:512, 0:512])
        for kc in range(_KC):
            (qB if kc % 2 else qC).dma_start(
                wv_sb[kc], wv[kc * 128:(kc + 1) * 128, :])
        # B: rest of X (needed from ec4 of quarter 0 onwards), then
        # head-chunk-1 q/k columns and wo
        qB.dma_start(xt_sb[0][:, 512:_N], xt[0:128, 512:_N])
        qC.dma_start(xt_sb[1][:, 512:_N], xt[128:256, 512:_N])
        qB.dma_start(xt_sb[2][:, 512:_N], xt[256:384, 512:_N])
        qC.dma_start(xt_sb[3][:, 512:_N], xt[384:512, 512:_N])
        for kc in range(_KC):
            qB.dma_start(wq_sb[kc][:, 128:_HL],
                         wq[kc * 128:(kc + 1) * 128, 128:_HL])
            qC.dma_start(wk_sb[kc][:, 128:_HL],
                         wk[kc * 128:(kc + 1) * 128, 128:_HL])
        qB.dma_start(wo_sb[0], wo[0:64, :])
        qC.dma_start(wo_sb[1], wo[64:128, :])
        qB.dma_start(wo_sb[2], wo[128:192, :])
        qC.dma_start(wo_sb[3], wo[192:256, :])

        ones11 = weights.tile([1, 1], f32, tag="ones11", name="ones11")
        nc.vector.memset(ones11, 1.0)

        # Warm the PE clock + preload the Exp table while input DMAs land.
        dummy = xtp.tile([128, 512], bf16, tag="dummy", name="dummy")
        nc.vector.memset(dummy, 0.0)
        dumact = xtp.tile([1, 8], bf16, tag="dumact", name="dumact")
        psw = psA.tile([128, 512], f32, tag="mm")
        nc.tensor.matmul(psw, lhsT=dummy[:, 0:128], rhs=dummy,
                         start=True, stop=True)
        nc.scalar.activation(dumact, psw[0:1, 0:8], Exp)
        for _ in range(4):
            psw = psA.tile([128, 512], f32, tag="mm")
            nc.tensor.matmul(psw, lhsT=dummy[:, 0:128], rhs=dummy,
                             start=True, stop=True)

        # ---- qkv projection pieces --------------------------------------
        qt_sb = [None, None]
        kt_sb = [None, None]

        def project_qk_piece(name, wsb, lst, hc, tp, pool=None, tag="mo"):
            if lst[hc] is None:
                lst[hc] = qkp.tile([128, _N], bf16, tag=f"{name}{hc}",
                                   name=f"{name}{hc}")
            dst = lst[hc]
            ps = (pool or psB).tile([128, 512], f32, tag=tag)
            for kc in range(_KC):
                nc.tensor.matmul(
                    ps,
                    lhsT=wsb[kc][:, hc * 128:(hc + 1) * 128],
                    rhs=xt_sb[kc][:, tp * 512:(tp + 1) * 512],
                    start=(kc == 0), stop=(kc == _KC - 1),
                )
            nc.vector.tensor_copy(dst[:, tp * 512:(tp + 1) * 512], ps)

        # V augmented with a ones column: va[t][:, h, 0:64] = V, [..., 64]=1
        va_sb = []
        for t in range(_TC):
            va_sb.append(vap.tile([128, _HPC, 65], bf16, tag=f"va{t}",
                                  name=f"va{t}"))

        def v_piece(t, pool=None, tag="mo"):
            va = va_sb[t]
            nc.gpsimd.memset(va[:, :, 64:65], 1.0)
            ps = (pool or psB).tile([128, _HL], f32, tag=tag)
            for kc in range(_KC):
                nc.tensor.matmul(
                    ps,
                    lhsT=xt_sb[kc][:, t * 128:(t + 1) * 128],
                    rhs=wv_sb[kc],
                    start=(kc == 0), stop=(kc == _KC - 1),
                )
            nc.vector.tensor_copy(
                va[:, :, 0:64], ps.rearrange("p (h d) -> p h d", h=_HPC))

        # Minimum serial prefix before attention: qt0/kt0 piece 0 (the
        # score prime right after these is emitted in the priming block
        # below, before the upfront V pieces, so the first exp does not
        # wait on V).
        V_UPFRONT = 4
        project_qk_piece("qt", wq_sb, qt_sb, 0, 0, pool=psA, tag="mm")
        project_qk_piece("kt", wk_sb, kt_sb, 0, 0, pool=psA, tag="mm")
        for t in range(V_UPFRONT):
            v_piece(t, pool=psA if t % 2 == 0 else psB,
                    tag="mm" if t % 2 == 0 else "mo")

        def mkv(t):
            return lambda: v_piece(t)

        def mkp(name, wsb, lst, hc, tp):
            return lambda: project_qk_piece(name, wsb, lst, hc, tp)

        # pending: drained one op per ec slot (popped BEFORE the score
        # prefetch, so a piece's matmuls always precede the first score
        # that reads it on the in-order PE queue).  Deadlines in quarter
        # 0: qt piece tp by slot 4*tp-2, va[t] by slot t-1, kt tp1 by
        # slot 14; kt tp2/tp3 during later quarters.
        pending = [
            mkp("qt", wq_sb, qt_sb, 0, 1), mkv(4), mkv(5),
            mkp("qt", wq_sb, qt_sb, 0, 2), mkv(6), mkv(7),
            mkp("qt", wq_sb, qt_sb, 0, 3), mkv(8), mkv(9),
            mkp("kt", wk_sb, kt_sb, 0, 1), mkv(10), mkv(11), mkv(12),
            mkv(13), mkv(14), mkv(15),
            mkp("kt", wk_sb, kt_sb, 0, 2),
        ]
        # pair 1 projections + kt tp3, drained at ec%3 during quarters 1-3
        pending_slow = [mkp("kt", wk_sb, kt_sb, 0, 3)]
        for tp in range(_N // 512):
            pending_slow.append(mkp("kt", wk_sb, kt_sb, 1, tp))
            pending_slow.append(mkp("qt", wq_sb, qt_sb, 1, tp))

        # ---- attention (paired heads) with pipelined out-projection ------
        acc = []
        for t in range(_TC):
            acc.append(outp.tile([128, _DIM], f32, tag=f"acc{t}",
                                 name=f"acc{t}"))
        # per (pair, head-in-pair): [65, N] attention rows; row 64 = Z
        at_sb = [[None, None], [None, None]]
        zrec = [None] * _HPC

        def outproj_chunk(pair, t, store, wide=False):
            h0, h1 = 2 * pair, 2 * pair + 1
            tsl = slice(t * 128, (t + 1) * 128)
            ps0 = psB.tile([128, _DIM], f32, tag="mo")
            ps1 = (psOp if wide else psB).tile(
                [128, _DIM], f32, tag="po" if wide else "mo")
            nc.tensor.matmul(ps0, lhsT=at_sb[pair][0][0:64, tsl],
                             rhs=wo_sb[h0], start=True, stop=True)
            nc.tensor.matmul(ps1, lhsT=at_sb[pair][1][0:64, tsl],
                             rhs=wo_sb[h1], start=True, stop=True)
            if h0 == 0:
                nc.vector.tensor_scalar_mul(acc[t], ps0, zrec[h0][:, t:t + 1])
            else:
                nc.vector.scalar_tensor_tensor(
                    out=acc[t], in0=ps0, scalar=zrec[h0][:, t:t + 1],
                    in1=acc[t], op0=Alu.mult, op1=Alu.add,
                )
            nc.vector.scalar_tensor_tensor(
                out=acc[t], in0=ps1, scalar=zrec[h1][:, t:t + 1],
                in1=acc[t], op0=Alu.mult, op1=Alu.add,
            )
            if store:
                (nc.gpsimd if t % 2 else nc.sync).dma_start(
                    out[tsl, :], acc[t])

        NQ = 4                      # n-quarters; po = [65, 512] = 1 bank

        def score_pair(pair, q, ec):
            # two heads' score chunks on disjoint PE row groups; ONE psum
            # tile [128, 1024] (h0 cols 0:512, h1 cols 512:1024)
            ncol = q * 512
            ps = psA.tile([128, 1024], f32, tag="mm")
            nc.tensor.matmul(
                ps[:, 0:512],
                lhsT=qt_sb[pair][0:64, ec * 128:(ec + 1) * 128],
                rhs=kt_sb[pair][0:64, ncol:ncol + 512],
                start=True, stop=True,
            )
            nc.tensor.matmul(
                ps[:, 512:1024],
                lhsT=qt_sb[pair][64:128, ec * 128:(ec + 1) * 128],
                rhs=kt_sb[pair][64:128, ncol:ncol + 512],
                start=True, stop=True,
            )
            return ps

        def mkz(h, zrow, q, qs, eng):
            # 1/Z columns via a DRAM bounce: [1, 512] z row -> [128, 4]
            def zchain():
                zd = zdp.tile([1, 512], f32, tag=f"zd{h % 2}")
                eng.dma_start(zd, zrow[0:1, qs])
                zcol = zp.tile([128, NQ], f32, tag=f"zcol{h % 2}")
                eng.dma_start(
                    zcol, zd.rearrange("o (j p) -> (o p) j", p=128))
                nc.vector.reciprocal(
                    zrec[h][:, q * NQ:(q + 1) * NQ], zcol)
            return zchain

        def mkz_pe(h, zrow, q):
            # tail variant: PE is idle after the last exp; use tensor-
            # engine transposes instead of the DMA bounce
            def zchain():
                pz = psB.tile([128, NQ], f32, tag="mo")
                for j in range(NQ):
                    jj = q * NQ + j
                    nc.tensor.transpose(
                        pz[:, j:j + 1],
                        zrow[0:1, jj * 128:(jj + 1) * 128], ones11)
                nc.vector.reciprocal(
                    zrec[h][:, q * NQ:(q + 1) * NQ], pz)
            return zchain

        ps_q = [None, None]         # prefetched score psums, slots ec%2
        ps_q[0] = score_pair(0, 0, 0)
        ps_q[1] = score_pair(0, 0, 1)
        for pair in range(2):
            if pair == 1:
                while pending_slow:
                    pending_slow.pop(0)()
            h0, h1 = 2 * pair, 2 * pair + 1
            for hi, h in enumerate((h0, h1)):
                at_sb[pair][hi] = atp.tile([64, _N], bf16, tag=f"at{pair}{hi}",
                                           name=f"at{pair}{hi}")
                zrec[h] = zp.tile([128, _TC], f32, tag=f"zrec{h}",
                                  name=f"zrec{h}", bufs=1)
            zrow0 = zp.tile([1, _N], f32, tag=f"zrow{h0}", name=f"zrow{h0}",
                            bufs=1)
            zrow1 = zp.tile([1, _N], f32, tag=f"zrow{h1}", name=f"zrow{h1}",
                            bufs=1)
            for q in range(NQ):
                ncol = q * 512
                qs = slice(ncol, ncol + 512)
                po0 = psOp.tile([65, 512], f32, tag="po")
                po1 = psOp.tile([65, 512], f32, tag="po")
                for ec in range(_TC):
                    if ps_q[ec % 2] is None:
                        ps_q[ec % 2] = score_pair(pair, q, ec)
                    ps = ps_q[ec % 2]
                    pt = ptp.tile([128, 1024], bf16, tag="pt")
                    nc.scalar.activation(pt, ps, Exp, scale=_SCALE)
                    ps_q[ec % 2] = None
                    # deferred work first, so its PE ops precede the
                    # prefetched scores that may depend on them
                    if pending:
                        pending.pop(0)()
                    elif pending_slow and ec % 3 == 0:
                        pending_slow.pop(0)()
                    # prefetch scores 2 chunks ahead (same pair; crosses
                    # into the next quarter / next pair's first quarter)
                    pec = ec + 2
                    if pec < _TC:
                        ps_q[pec % 2] = score_pair(pair, q, pec)
                    elif q + 1 < NQ:
                        ps_q[pec % 2] = score_pair(pair, q + 1, pec - _TC)
                    elif pair == 0:
                        ps_q[pec % 2] = score_pair(1, 0, pec - _TC)
                    nc.tensor.matmul(
                        po0[0:65, :], lhsT=va_sb[ec][:, h0, :],
                        rhs=pt[:, 0:512],
                        start=(ec == 0), stop=(ec == _TC - 1),
                    )
                    nc.tensor.matmul(
                        po1[0:65, :], lhsT=va_sb[ec][:, h1, :],
                        rhs=pt[:, 512:1024],
                        start=(ec == 0), stop=(ec == _TC - 1),
                    )
                # drain the quarter: one [65, 512] copy per head (row 64
                # is the Z row).  In the final quarter ACT is idle after
                # the last exp, so the copies run there.
                last_q = (pair == 1 and q == NQ - 1)
                if last_q:
                    nc.vector.tensor_copy(zrow0[:, qs], po0[64:65, :])
                    nc.scalar.activation(at_sb[pair][0][:, qs],
                                         po0[0:64, :], Copy)
                    nc.vector.tensor_copy(zrow1[:, qs], po1[64:65, :])
                    nc.scalar.activation(at_sb[pair][1][:, qs],
                                         po1[0:64, :], Copy)
                    pending.append(mkz_pe(h0, zrow0, q))
                    pending.append(mkz_pe(h1, zrow1, q))
                else:
                    nc.vector.tensor_copy(at_sb[pair][0][:, qs], po0[0:64, :])
                    nc.vector.tensor_copy(zrow0[:, qs], po0[64:65, :])
                    nc.vector.tensor_copy(at_sb[pair][1][:, qs], po1[0:64, :])
                    nc.vector.tensor_copy(zrow1[:, qs], po1[64:65, :])
                    pending.append(mkz(h0, zrow0, q, qs, nc.sync))
                    pending.append(mkz(h1, zrow1, q, qs, nc.gpsimd))
                    # give the z DMA chains a head start so the out-proj
                    # RMWs don't stall on zrec mid-quarter
                    pending.append(lambda: None)
                    pending.append(lambda: None)
                for j in range(NQ):
                    t = q * NQ + j

                    def mk(pair, t, store, wide):
                        return lambda: outproj_chunk(pair, t, store, wide)

                    pending.append(mk(pair, t, pair == 1, last_q))
        while pending:
            pending.pop(0)()


def _build():
    if "/opt/trn_rl_repo" not in sys.path:
        sys.path.insert(0, "/opt/trn_rl_repo")
    from concourse import bacc, mybir
    import concourse.tile as tile

    dt = mybir.dt
    nc = bacc.Bacc("TRN2", target_bir_lowering=False, debug=False,
                   num_devices=_NCORES)
    xt = nc.dram_tensor("xt", [_DIM, _N], dt.bfloat16, kind="ExternalInput").ap()
    wq = nc.dram_tensor("wq", [_DIM, _HL], dt.bfloat16, kind="ExternalInput").ap()
    wk = nc.dram_tensor("wk", [_DIM, _HL], dt.bfloat16, kind="ExternalInput").ap()
    wv = nc.dram_tensor("wv", [_DIM, _HL], dt.bfloat16, kind="ExternalInput").ap()
    wo = nc.dram_tensor("wo", [_HL, _DIM], dt.bfloat16, kind="ExternalInput").ap()
    out = nc.dram_tensor("out", [_N, _DIM], dt.float32, kind="ExternalOutput").ap()

    with tile.TileContext(nc) as tc:
        _emit(tc, xt, wq, wk, wv, wo, out, mybir)
    nc.compile()
    return nc


def _get_nc():
    if "nc" not in _cache:
        _cache["nc"] = _build()
    return _cache["nc"]


def _shard_inputs(X, W_qkv, W_out):
    import ml_dtypes
    bf16 = ml_dtypes.bfloat16
    in_maps = []
    for c in range(_NCORES):
        b, g = c // 2, c % 2
        cols = slice(g * _HL, (g + 1) * _HL)
        in_maps.append({
            "xt": np.ascontiguousarray(X[b].T).astype(bf16),
            "wq": W_qkv[:, 0 * _DIM:][:, cols].astype(bf16),
            "wk": W_qkv[:, 1 * _DIM:][:, cols].astype(bf16),
            "wv": W_qkv[:, 2 * _DIM:][:, cols].astype(bf16),
            "wo": W_out[g * _HL:(g + 1) * _HL, :].astype(bf16),
        })
    return in_maps


def _run(inputs, trace=False):
    if "/opt/trn_rl_repo" not in sys.path:
        sys.path.insert(0, "/opt/trn_rl_repo")
    from concourse.bass_utils import run_bass_kernel_spmd

    X = np.asarray(inputs["X"], dtype=np.float32)
    W_qkv = np.asarray(inputs["W_qkv"], dtype=np.float32)
    W_out = np.asarray(inputs["W_out"], dtype=np.float32)
    b_out = np.asarray(inputs["b_out"], dtype=np.float32)

    nc = _get_nc()
    in_maps = _shard_inputs(X, W_qkv, W_out)
    res = run_bass_kernel_spmd(nc, in_maps, list(range(_NCORES)), trace=trace)

    out = np.empty((_B, _N, _DIM), dtype=np.float32)
    for b in range(_B):
        out[b] = res.results[2 * b]["out"] + res.results[2 * b + 1]["out"] + b_out
    return out, res.exec_time_ns


def kernel(**inputs) -> np.ndarray:
    out, _ = _run(inputs, trace=False)
    return out
